# revision 96
# baseline (speedup 1.0000x reference)
"""Trainium2 Bass kernel for nn_MoEClassifier (6-layer transformer backbone +
softmax-routed MoE head), SPMD over 8 NeuronCores.

Sharding: data-parallel backbone (2 of 16 batch rows per core, params
replicated), expert-parallel MoE head (core c owns expert c) glued by an
on-device AllGather of the pooled features; the host sums the 8 per-expert
partial outputs.

v2 rewrite vs the f32r baseline:
- all weights + matmul activations in bf16 (rel-err budget is 2e-2, baseline
  was at 4e-4); weights converted on host, one large DMA per weight per layer
- no DVE reciprocal anywhere on the hot path: 1/z computed as exp(-ln z) on
  the Scalar engine (table-accurate; z > 0 always)
- LayerNorm pipelined per token-half with per-(hc,tq) hT tiles so QKV matmuls
  start as soon as their chunk is normalized
- attention softmax denominators batched per batch-row into one [NH,512] tile
- FFN W2 loop runs of-outer/m-inner so only ~3 ffT chunks are live
- weights loaded once per layer (both batch rows / token halves share them)
"""

import numpy as np
import ml_dtypes

import concourse.bass as bass
import concourse.mybir as mybir
from concourse.bass_utils import run_bass_kernel_spmd
from concourse.tile import TileContext
from concourse.vector_clock import ScopedClock

B, S, V, H, L, NH, FF, E, FE, C = 16, 512, 30522, 768, 6, 8, 3072, 8, 3072, 1000
HD = H // NH          # 96
NCORES = 8
BL = B // NCORES      # 2 batch rows per core
T = BL * S            # 1024 tokens per core
HC = H // 128         # 6 hidden chunks
FFC = FF // 128       # 24 ffn chunks
EPS = 1e-5

f32 = mybir.dt.float32
f32r = mybir.dt.float32r
bf16 = mybir.dt.bfloat16
AF = mybir.ActivationFunctionType
AX = mybir.AxisListType
OP = mybir.AluOpType
ts = bass.ts

MAX_WAITS = 1


class PatchedTileContext(TileContext):
    """Workaround for this walrus build's 1-sync-wait-per-instruction limit:
    split excess semaphore waits onto single-wait NOPs inserted immediately
    before the owning instruction (same engine, same program point)."""

    def _split_excess_waits(self, ordered):
        nc = self.nc
        for bb_name, insts in list(ordered.items()):
            new_list = []
            changed = False
            for inst in insts:
                si = getattr(inst, "sync_info", None)
                if si is not None and len(si.on_wait) > MAX_WAITS:
                    waits = list(si.on_wait)
                    movable = [
                        w for w in waits
                        if w.sync_type == "semaphore" and w.wait_mode == "sem-ge-imm"
                    ]
                    n_fixed = len(waits) - len(movable)
                    keep_n = max(0, MAX_WAITS - n_fixed)
                    n_over = max(0, len(movable) - keep_n)
                    overflow = movable[:n_over]
                    keep = [w for w in waits if w not in overflow]
                    assert len(keep) <= MAX_WAITS, (
                        f"cannot legalize waits on {inst.name}"
                    )
                    for w in overflow:
                        nop = mybir.InstNoOp(
                            name=f"I-{nc.next_id()}",
                            sync_info=mybir.SyncInfo(on_wait=[w], on_update=[]),
                            bass_nofuse=True,
                            engine=inst.engine,
                        )
                        new_list.append(nop)
                    inst.sync_info = mybir.SyncInfo(
                        on_wait=keep, on_update=list(si.on_update)
                    )
                    changed = True
                new_list.append(inst)
            if changed:
                ordered[bb_name] = new_list

    def _lower_ordered_insts(self, ordered):
        self._split_excess_waits(ordered)
        return super()._lower_ordered_insts(ordered)

    def _drain_and_barrier(self, tick_clock, wait_clock):
        nops = [self.nc.sync.nop(nofuse=True, hint=f"dw_{i}") for i in range(40)]
        drain_inst = self.nc.sync.drain()
        wait_clock.add_sem_waits(
            drain_inst.ins, ScopedClock({None: tick_clock.global_clock})
        )
        si = drain_inst.ins.sync_info
        if si is not None and len(si.on_wait) > 1:
            waits = list(si.on_wait)
            rest, keep = waits[:-1], waits[-1:]
            assert len(rest) <= len(nops)
            for nop_bi, w in zip(nops, rest):
                nop_bi.ins.sync_info = mybir.SyncInfo(on_wait=[w], on_update=[])
            drain_inst.ins.sync_info = mybir.SyncInfo(
                on_wait=keep, on_update=list(si.on_update)
            )
        self.nc.all_engine_barrier()
        assert self.sems is not None
        popped = self.nc._tile_sem_poison_stack.pop()
        assert popped is self._sem_poison
        self.nc.clear_and_free_semaphores(list(self.sems.allocated().values()))
        self.nc.all_engine_barrier()


def _r(ap):
    return ap.bitcast(f32r)


def build_program(n_layers=L, debug=False):
    nc = bass.Bass()

    x0T_d = nc.dram_tensor("x0T", [H, T], f32, kind="ExternalInput")
    wqkv_d = nc.dram_tensor("wqkv", [n_layers, H, 3 * H], bf16, kind="ExternalInput")
    # host pre-arranged [L, HD, NH, H]
    wo_d = nc.dram_tensor("wo", [n_layers, HD, NH, H], bf16, kind="ExternalInput")
    w1_d = nc.dram_tensor("w1", [n_layers, H, FF], bf16, kind="ExternalInput")
    w2_d = nc.dram_tensor("w2", [n_layers, FF, H], bf16, kind="ExternalInput")
    wr_d = nc.dram_tensor("wr", [H, E], bf16, kind="ExternalInput")
    we1_d = nc.dram_tensor("we1m", [H, FE], bf16, kind="ExternalInput")
    we2_d = nc.dram_tensor("we2m", [FE, C], bf16, kind="ExternalInput")
    maske_d = nc.dram_tensor("maske", [B, E], f32, kind="ExternalInput")
    # consts: col0 = -1/H, col1 = 1.0, col2.. = 1.0 row for broadcasts
    ones_d = nc.dram_tensor("ones", [128, 128], f32, kind="ExternalInput")
    # col0 = -1/H, col1 = EPS
    negh_d = nc.dram_tensor("negh", [128, 2], f32, kind="ExternalInput")
    neghb_d = nc.dram_tensor("neghb", [128, 1], bf16, kind="ExternalInput")
    onesb_d = nc.dram_tensor("onesb", [128, 128], bf16, kind="ExternalInput")
    id128_d = nc.dram_tensor("id128", [128, 128], f32, kind="ExternalInput")
    id16_d = nc.dram_tensor("id16", [16, 16], f32, kind="ExternalInput")
    y_d = nc.dram_tensor("y", [B, C], f32, kind="ExternalOutput")

    dbg = {}
    if debug:
        for name, shape in [("dbg_h1", [H, T]), ("dbg_xa", [H, T]),
                            ("dbg_x1", [H, T]), ("dbg_pool", [BL, H]),
                            ("dbg_gate", [B, E])]:
            dbg[name] = nc.dram_tensor(name, shape, f32, kind="ExternalOutput")

    from contextlib import ExitStack

    lp = nc.allow_low_precision(reason="bf16 matmuls + f32r stats")
    lp.__enter__()
    with PatchedTileContext(nc) as tc:
        with tc.tile_pool(name="sbc", bufs=1) as sbc:
            bk = ExitStack()
            sbw = bk.enter_context(tc.tile_pool(name="sbw", bufs=1))
            sbw3 = bk.enter_context(tc.tile_pool(name="sbw3", bufs=2))
            sbh = bk.enter_context(tc.tile_pool(name="sbh", bufs=12))
            sba = bk.enter_context(tc.tile_pool(name="sba", bufs=1))
            sbs = bk.enter_context(tc.tile_pool(name="sbs", bufs=2))
            sbr = bk.enter_context(tc.tile_pool(name="sbr", bufs=4))
            sbx = bk.enter_context(tc.tile_pool(name="sbx", bufs=7))
            sbf = bk.enter_context(tc.tile_pool(name="sbf", bufs=9))
            sbhf = bk.enter_context(tc.tile_pool(name="sbhf", bufs=3))
            ps_st = bk.enter_context(
                tc.tile_pool(name="ps_st", bufs=2, space="PSUM"))
            ps_bc = bk.enter_context(
                tc.tile_pool(name="ps_bc", bufs=2, space="PSUM"))

            # ---------------- constants
            negh = sbc.tile([128, 1], bf16, tag="negh")     # -1/H
            nc.sync.dma_start(negh[:], neghb_d[:])
            epsc = sbc.tile([128, 1], f32, tag="epsc")      # EPS
            nc.sync.dma_start(epsc[:], negh_d[:, 1:2])
            # ones rows at partitions 0 and 32 (broadcast-matmul lhsT must
            # share base_partition with its rhs row)
            onesrow = sbc.tile([33, 128], bf16, tag="onesrow")
            nc.sync.dma_start(onesrow[:], onesb_d[0:33, :])
            onesb = sbc.tile([128, 32], bf16, tag="onesb")    # bf16 1.0
            nc.sync.dma_start(onesb[:], onesb_d[:, 0:32])
            id128 = sbc.tile([128, 128], f32, tag="id128")
            nc.sync.dma_start(id128[:], id128_d[:])

            x = sbc.tile([128, HC, T], f32, tag="x")
            for hc in range(HC):
                nc.sync.dma_start(
                    x[:, hc, :],
                    x0T_d.rearrange("(hc p) t -> p hc t", p=128)[:, hc, :])

            def layer_norm_half(tq, hts, pool=None, dt=bf16):
                """LN over hidden for token half tq: appends 6 normalized
                chunk tiles to hts. Uses the static ps_st/ps_bc PSUM pools;
                all matmuls bf16 (stats from a bf16 copy of x)."""
                pool_ = pool if pool is not None else sbh
                sx = ps_st.tile([1, 512], f32, tag="st")
                sq = ps_st.tile([1, 512], f32, tag="st")
                xbs = []
                for hc in range(HC):
                    xb = sbx.tile([128, 512], bf16, tag="xb")
                    nc.any.tensor_copy(xb[:], x[:, hc, ts(tq, 512)])
                    xbs.append(xb)
                    sqc = sbs.tile([128, 512], bf16, tag="sqc")
                    nc.scalar.activation(sqc[:], x[:, hc, ts(tq, 512)], AF.Square,
                                         scale=float(1.0 / np.sqrt(H)))
                    nc.tensor.matmul(sx[:], negh[:], xb[:],
                                     start=(hc == 0), stop=(hc == HC - 1))
                    nc.tensor.matmul(sq[:], onesb[:, 0:1], sqc[:],
                                     start=(hc == 0), stop=(hc == HC - 1))
                # row math: nmu = sx (= -mu); var = sq - nmu^2; r = exp(-.5 ln(var+eps))
                nmu = sbr.tile([1, 512], bf16, tag="rowb")
                nc.vector.tensor_copy(nmu[:], sx[:])
                # heartbeat matmuls: keep the PE's HAM activity window busy
                # through the serial row chain (re-throttle fires after one
                # fully idle ~3.4us window); hb1 runs right after the stats,
                # hb2 after nmu — their outputs are overwritten by nb below
                nb = ps_bc.tile([128, 512], f32, tag="bc")
                nc.tensor.matmul(nb[0:1, :], onesb[:, 0:1], xbs[5][:],
                                 start=True, stop=True)
                nc.tensor.matmul(nb[:], onesrow[0:1, :], nmu[:],
                                 start=True, stop=True)
                mu2 = sbr.tile([1, 512], bf16, tag="rowb")
                nc.vector.tensor_tensor(mu2[:], nmu[:], nmu[:], OP.mult)
                var = sbr.tile([1, 512], bf16, tag="rowb")
                nc.vector.tensor_tensor(var[:], sq[:], mu2[:], OP.subtract)
                rb = ps_bc.tile([128, 512], f32, tag="bc")
                nc.tensor.matmul(rb[:], onesrow[0:1, :], mu2[:],
                                 start=True, stop=True)
                nc.tensor.matmul(rb[:], onesrow[0:1, :], var[:],
                                 start=True, stop=True)
                lnv = sbr.tile([1, 512], f32, tag="row")
                nc.scalar.activation(lnv[:], var[:], AF.Ln, bias=epsc[0:1])
                rstd = sbr.tile([1, 512], bf16, tag="rowb")
                nc.scalar.activation(rstd[:], lnv[:], AF.Exp, scale=-0.5)
                nc.tensor.matmul(nb[:], onesrow[0:1, :], nmu[:],
                                 start=True, stop=True)
                nc.tensor.matmul(rb[:], onesrow[0:1, :], rstd[:],
                                 start=True, stop=True)
                nbS = sbs.tile([128, 512], bf16, tag="nbS")
                nc.scalar.activation(nbS[:], nb[:], AF.Copy)
                rbS = sbs.tile([128, 512], bf16, tag="nbS")
                nc.scalar.activation(rbS[:], rb[:], AF.Copy)
                for hc in range(HC):
                    tmp = sbs.tile([128, 512], bf16, tag="tmp")
                    nc.vector.tensor_tensor(tmp[:], xbs[hc][:], nbS[:], OP.add)
                    ht = pool_.tile([128, 512], dt, tag="hT")
                    nc.vector.tensor_tensor(ht[:], tmp[:], rbS[:], OP.mult)
                    hts.append(ht)

            pooledT = sbc.tile([128, HC, BL], f32, tag="pooledT")
            hts = []
            layer_norm_half(0, hts)
            layer_norm_half(1, hts)
            for l in range(n_layers):
                # ---------------- weights for this layer (one DMA each)
                wqkv = sbw.tile([128, HC, 3 * H], bf16, tag="wqkv")
                nc.sync.dma_start(
                    wqkv[:], wqkv_d[l].rearrange("(hc p) m -> p hc m", p=128))
                wo_t = sbw.tile([HD, NH, H], bf16, tag="wo")
                nc.sync.dma_start(wo_t[:], wo_d[l])

                # hts for this layer were produced at the tail of the previous
                # layer's FFN (software-pipelined LN1)
                for b2 in range(BL):
                    ht_b = hts[b2 * HC:(b2 + 1) * HC]
                    qT = sba.tile([HD, NH, 512], bf16, tag="qT")
                    kT = sba.tile([HD, NH, 512], bf16, tag="kT")
                    v_aug = sba.tile([128, 4, NH, HD + 1], bf16, tag="vaug")
                    nc.vector.tensor_copy(
                        v_aug[:, :, :, HD:],
                        onesb[:].rearrange("p (a b c) -> p a b c", a=4, b=NH, c=1))
                    with tc.tile_pool(name=f"psqkv_{l}_{b2}", bufs=4,
                                      space="PSUM") as ps:
                        for h in range(NH):
                            pq = ps.tile([HD, 512], f32, tag="mm")
                            pk = ps.tile([HD, 512], f32, tag="mm")
                            for hc in range(HC):
                                rhs = ht_b[hc][:]
                                nc.tensor.matmul(
                                    pq[:], wqkv[:, hc, h * HD:(h + 1) * HD], rhs,
                                    start=(hc == 0), stop=(hc == HC - 1))
                                nc.tensor.matmul(
                                    pk[:], wqkv[:, hc, H + h * HD:H + (h + 1) * HD],
                                    rhs, start=(hc == 0), stop=(hc == HC - 1))
                            nc.vector.tensor_copy(qT[:, h, :], pq[:])
                            nc.scalar.activation(kT[:, h, :], pk[:], AF.Copy)
                        # V: token-major via lhsT = hT chunks
                        for tt in range(4):
                            pv0 = ps.tile([128, 384], f32, tag="mm")
                            pv1 = ps.tile([128, 384], f32, tag="mm")
                            for hc in range(HC):
                                lhs = ht_b[hc][:, ts(tt, 128)]
                                nc.tensor.matmul(
                                    pv0[:], lhs, wqkv[:, hc, 2 * H:2 * H + 384],
                                    start=(hc == 0), stop=(hc == HC - 1))
                                nc.tensor.matmul(
                                    pv1[:], lhs, wqkv[:, hc, 2 * H + 384:3 * H],
                                    start=(hc == 0), stop=(hc == HC - 1))
                            nc.vector.tensor_copy(
                                v_aug[:, tt, 0:4, :HD],
                                pv0[:].rearrange("p (h d) -> p h d", h=4))
                            nc.vector.tensor_copy(
                                v_aug[:, tt, 4:8, :HD],
                                pv1[:].rearrange("p (h d) -> p h d", h=4))

                    # attention; 1/z = exp(-ln z) on ScalarE, batched per
                    # head-pair to halve the ACT denominator work
                    oT = sba.tile([HD, NH, 512], bf16, tag="oT")
                    with tc.tile_pool(name=f"psat_{l}_{b2}", bufs=2,
                                      space="PSUM") as ps:
                        for h in range(NH):
                            expT = sbs.tile([128, 4, 512], bf16, tag="expT")
                            for tk in range(4):
                                psc = ps.tile([128, 512], f32, tag="sc")
                                nc.tensor.matmul(
                                    psc[:], kT[:, h, ts(tk, 128)], qT[:, h, :],
                                    start=True, stop=True)
                                nc.scalar.activation(
                                    expT[:, tk, :], psc[:], AF.Exp,
                                    scale=float(1.0 / np.sqrt(HD)))
                            po = ps.tile([HD + 1, 512], f32, tag="o")
                            for tk in range(4):
                                nc.tensor.matmul(po[:], v_aug[:, tk, h, :],
                                                 expT[:, tk, :],
                                                 start=(tk == 0), stop=(tk == 3))
                            lnz = sbr.tile([1, 512], f32, tag="row")
                            nc.scalar.activation(lnz[:], po[HD:HD + 1, :], AF.Ln)
                            rz = sbr.tile([1, 512], bf16, tag="rowb")
                            nc.scalar.activation(rz[:], lnz[:], AF.Exp,
                                                 scale=-1.0)
                            prb = ps_bc.tile([HD, 512], f32, tag="bc")
                            nc.tensor.matmul(prb[:], onesrow[0:1, :HD], rz[:],
                                             start=True, stop=True)
                            rbS = sbs.tile([HD, 512], bf16, tag="rbS")
                            nc.vector.tensor_copy(rbS[:], prb[:])
                            nc.vector.tensor_tensor(oT[:, h, :], po[:HD, :],
                                                    rbS[:], OP.mult)

                    # Wo + residual
                    with tc.tile_pool(name=f"pswo_{l}_{b2}", bufs=2,
                                      space="PSUM") as ps:
                        for m in range(HC):
                            pwo = ps.tile([128, 512], f32, tag="wo")
                            for h in range(NH):
                                nc.tensor.matmul(pwo[:], wo_t[:, h, ts(m, 128)],
                                                 oT[:, h, :],
                                                 start=(h == 0),
                                                 stop=(h == NH - 1))
                            nc.vector.tensor_tensor(x[:, m, ts(b2, 512)],
                                                    x[:, m, ts(b2, 512)],
                                                    pwo[:], OP.add)

                if debug and l == 0:
                    nc.sync.dma_start(
                        dbg["dbg_xa"].rearrange("(hc p) t -> p hc t", p=128), x[:])

                # ---------------- LN2 + FFN (LN1 of layer l+1 interleaved)
                hts_next = []
                for tq in range(2):
                    hts2 = []
                    layer_norm_half(tq, hts2)
                    with tc.tile_pool(name=f"psf1_{l}_{tq}", bufs=2,
                                      space="PSUM") as psw1, \
                         tc.tile_pool(name=f"psf2_{l}_{tq}", bufs=2,
                                      space="PSUM") as psx2:
                        for t4 in range(4):
                            w1_4 = sbw3.tile([128, HC, 768], bf16, tag="w1h")
                            nc.sync.dma_start(
                                w1_4[:],
                                w1_d[l].rearrange("(hc p) m -> p hc m",
                                                  p=128)[:, :, ts(t4, 768)])
                            w2_4 = sbw3.tile([128, 6, H], bf16, tag="w2h")
                            nc.sync.dma_start(
                                w2_4[:],
                                w2_d[l].rearrange(
                                    "(fc p) m -> p fc m",
                                    p=128)[:, t4 * 6:(t4 + 1) * 6, :])
                            ffTs = []
                            for of6 in range(6):
                                pf = psw1.tile([128, 512], f32, tag="w1")
                                for hc in range(HC):
                                    nc.tensor.matmul(
                                        pf[:], w1_4[:, hc, ts(of6, 128)],
                                        hts2[hc][:],
                                        start=(hc == 0), stop=(hc == HC - 1))
                                ffT = sbf.tile([128, 512], bf16, tag="ffT")
                                nc.scalar.activation(ffT[:], pf[:], AF.Gelu)
                                ffTs.append(ffT)
                            for m in range(HC):
                                px2 = psx2.tile([128, 512], f32, tag="x2")
                                for of6 in range(6):
                                    nc.tensor.matmul(
                                        px2[:], w2_4[:, of6, ts(m, 128)],
                                        ffTs[of6][:],
                                        start=(of6 == 0), stop=(of6 == 5))
                                nc.vector.tensor_tensor(x[:, m, ts(tq, 512)],
                                                        x[:, m, ts(tq, 512)],
                                                        px2[:], OP.add)
                    # next layer's LN1 for this token half: its stats/normalize
                    # interleave with the other half's FFN matmul stream
                    if l < n_layers - 1:
                        layer_norm_half(tq, hts_next)
                hts = hts_next
                if debug and l == 0:
                    nc.sync.dma_start(
                        dbg["dbg_x1"].rearrange("(hc p) t -> p hc t", p=128), x[:])

            # ---------------- final LN + pooling (f32 for gate fidelity)
            for tq in range(2):
                htf = []
                layer_norm_half(tq, htf, pool=sbhf, dt=f32)
                for hc in range(HC):
                    acc = sbr.tile([128, 1], f32, tag="poolacc")
                    nc.vector.reduce_sum(acc[:], htf[hc][:], axis=AX.X)
                    nc.vector.tensor_scalar_mul(pooledT[:, hc, tq:tq + 1],
                                                acc[:], 1.0 / S)
            pool_tok = sbc.tile([BL, H], f32, tag="pool_tok")
            with tc.tile_pool(name="pstr", bufs=2, space="PSUM") as ps:
                for hc in range(HC):
                    pt = ps.tile([BL, 128], f32, tag="tr")
                    nc.tensor.transpose(pt[:], pooledT[:, hc, :], id128[:])
                    nc.vector.tensor_copy(pool_tok[:, ts(hc, 128)], pt[:])
            if debug:
                nc.sync.dma_start(dbg["dbg_pool"][:], pool_tok[:])

            # release backbone pools so the head weights fit in SBUF
            bk.close()

            # ---------------- AllGather (in-context) + MoE head
            with tc.tile_pool(name="dcc", bufs=1, space="DRAM") as dcc, \
                 tc.tile_pool(name="hsb1", bufs=1) as hb1, \
                 tc.tile_pool(name="hsb4", bufs=4) as hb4:
                in_b = dcc.tile([BL, H], f32, tag="ccin")
                out_b = dcc.tile([B, H], f32, tag="ccout", addr_space="Shared")
                nc.sync.dma_start(in_b[:], pool_tok[:])
                nc.gpsimd.collective_compute(
                    "AllGather", OP.bypass,
                    replica_groups=[list(range(NCORES))],
                    ins=[in_b.opt()], outs=[out_b.opt()],
                )

                # weight DMAs (chunked so first matmuls start early; they
                # overlap the collective)
                we1 = hb1.tile([128, HC, FE], bf16, tag="we1")
                for fr in range(3):
                    nc.sync.dma_start(
                        we1[:, :, ts(fr, 1024)],
                        we1_d.rearrange("(hc p) m -> p hc m",
                                        p=128)[:, :, ts(fr, 1024)])
                we2 = hb1.tile([128, FFC, C], bf16, tag="we2")
                for fr in range(3):
                    nc.sync.dma_start(
                        we2[:, fr * 8:(fr + 1) * 8, :],
                        we2_d.rearrange("(fc p) m -> p fc m",
                                        p=128)[:, fr * 8:(fr + 1) * 8, :])
                wr_t = hb1.tile([128, HC, E], bf16, tag="wr")
                nc.sync.dma_start(wr_t[:],
                                  wr_d.rearrange("(hc p) e -> p hc e", p=128))
                id16 = hb1.tile([16, 16], f32, tag="id16")
                nc.sync.dma_start(id16[:], id16_d[:])
                maske = hb1.tile([B, E], f32, tag="maske")
                nc.sync.dma_start(maske[:], maske_d[:])

                pg = hb1.tile([B, H], f32, tag="pg")
                nc.gpsimd.dma_start(pg[:], out_b[:])
                paT = hb1.tile([128, HC, B], bf16, tag="paT")
                hps_cm = tc.tile_pool(name="hps", bufs=2, space="PSUM")
                ps = hps_cm.__enter__()
                for hc in range(HC):
                    pt = ps.tile([128, B], f32, tag="tr")
                    nc.tensor.transpose(pt[:], pg[:, ts(hc, 128)], id16[:])
                    nc.vector.tensor_copy(paT[:, hc, :], pt[:])
                # gate (token-major [B, E])
                pgl = ps.tile([B, E], f32, tag="gl")
                for hc in range(HC):
                    nc.tensor.matmul(pgl[:], paT[:, hc, :], wr_t[:, hc, :],
                                     start=(hc == 0), stop=(hc == HC - 1))
                gate = hb1.tile([B, E], f32, tag="gate")
                gmax = hb4.tile([B, 1], f32, tag="grow")
                nc.vector.reduce_max(gmax[:], pgl[:], axis=AX.X)
                ngmax = hb4.tile([B, 1], f32, tag="grow")
                nc.vector.tensor_scalar_mul(ngmax[:], gmax[:], -1.0)
                nc.scalar.activation(gate[:], pgl[:], AF.Exp, bias=ngmax[:])
                gsum = hb4.tile([B, 1], f32, tag="grow")
                nc.vector.reduce_sum(gsum[:], gate[:], axis=AX.X)
                grecip = hb4.tile([B, 1], f32, tag="grow")
                nc.vector.reciprocal(grecip[:], gsum[:])
                nc.vector.tensor_scalar_mul(gate[:], gate[:], grecip[:])
                if debug:
                    nc.sync.dma_start(dbg["dbg_gate"][:], gate[:])
                gcol = hb1.tile([B, 1], f32, tag="gcol")
                nc.vector.tensor_tensor(maske[:], gate[:], maske[:], OP.mult)
                nc.vector.reduce_sum(gcol[:], maske[:], axis=AX.X)

                # ehT = gelu(We1^T @ pooled_all) feature-major [FE, B]
                ehT = hb1.tile([128, FFC, B], bf16, tag="ehT")
                for fet in range(FFC):
                    pe_ = ps.tile([128, B], f32, tag="eh")
                    for hc in range(HC):
                        nc.tensor.matmul(pe_[:], we1[:, hc, ts(fet, 128)],
                                         paT[:, hc, :],
                                         start=(hc == 0), stop=(hc == HC - 1))
                    nc.scalar.activation(ehT[:, fet, :], pe_[:], AF.Gelu)
                # elog token-major [B, C] scaled by this expert's gate column
                y_sb = hb1.tile([B, C], f32, tag="y")
                for cn in range(2):
                    csz = C // 2
                    pel = ps.tile([B, csz], f32, tag="el")
                    for fet in range(FFC):
                        nc.tensor.matmul(pel[:], ehT[:, fet, :],
                                         we2[:, fet, ts(cn, csz)],
                                         start=(fet == 0), stop=(fet == FFC - 1))
                    nc.vector.tensor_scalar_mul(y_sb[:, ts(cn, csz)], pel[:],
                                                gcol[:])
                hps_cm.__exit__(None, None, None)
                nc.sync.dma_start(y_d[:], y_sb[:])

    lp.__exit__(None, None, None)
    return nc, dbg


_CACHE = {}


def _get_program(n_layers=L, debug=False):
    key = (n_layers, debug)
    if key not in _CACHE:
        _CACHE[key] = build_program(n_layers, debug)
    return _CACHE[key]


def prepare_inputs(inputs, n_layers=L):
    """Host-side shard prep: embedding gather, bf16 weight conversion,
    per-core slicing, asserts."""
    ids = np.asarray(inputs["input_ids"])
    mask = np.asarray(inputs["attention_mask"])
    assert (mask == 1).all(), "kernel assumes attention_mask == ones"
    for k in ("bqkv", "bo", "b1", "b2", "br", "be1", "be2",
              "ln1_b", "ln2_b", "lnf_b"):
        assert not np.any(np.asarray(inputs[k])), f"{k} must be zero"
    for k in ("ln1_g", "ln2_g", "lnf_g"):
        assert np.all(np.asarray(inputs[k]) == 1.0), f"{k} must be ones"

    bf = ml_dtypes.bfloat16
    tok = np.asarray(inputs["tok_emb"], np.float32)
    pos = np.asarray(inputs["pos_emb"], np.float32)
    x0 = tok[ids] + pos[None]                      # [B, S, H]
    wqkv = np.ascontiguousarray(
        np.asarray(inputs["Wqkv"], np.float32)[:n_layers]).astype(bf)
    wo = np.asarray(inputs["Wo"], np.float32)[:n_layers]
    # [L, H, H] -> [L, HD, NH, H]
    wo = np.ascontiguousarray(
        wo.reshape(n_layers, NH, HD, H).transpose(0, 2, 1, 3)).astype(bf)
    w1 = np.ascontiguousarray(
        np.asarray(inputs["W1"], np.float32)[:n_layers]).astype(bf)
    w2 = np.ascontiguousarray(
        np.asarray(inputs["W2"], np.float32)[:n_layers]).astype(bf)
    wr = np.ascontiguousarray(np.asarray(inputs["Wr"], np.float32)).astype(bf)
    we1 = np.asarray(inputs["We1"], np.float32)
    we2 = np.asarray(inputs["We2"], np.float32)
    id128 = np.eye(128, dtype=np.float32)
    id16 = np.eye(16, dtype=np.float32)
    negh = np.stack([np.full(128, -1.0 / H, np.float32),
                     np.full(128, EPS, np.float32)], axis=1)
    neghb = np.full((128, 1), -1.0 / H, np.float32).astype(bf)
    onesb = np.ones((128, 128), bf)

    in_maps = []
    for c in range(NCORES):
        rows = x0[c * BL:(c + 1) * BL]              # [BL, S, H]
        x0T = np.ascontiguousarray(rows.reshape(T, H).T)   # [H, T]
        maske = np.zeros((B, E), np.float32)
        maske[:, c] = 1.0
        in_maps.append({
            "x0T": x0T, "wqkv": wqkv, "wo": wo, "w1": w1, "w2": w2,
            "wr": wr, "we1m": np.ascontiguousarray(we1[c]).astype(bf),
            "we2m": np.ascontiguousarray(we2[c]).astype(bf),
            "maske": maske, "id128": id128, "id16": id16,
            "ones": np.ones((128, 128), np.float32),
            "negh": negh, "neghb": neghb, "onesb": onesb,
        })
    return in_maps


def kernel(**inputs):
    nc, _dbg = _get_program(L, debug=False)
    in_maps = prepare_inputs(inputs, L)
    res = run_bass_kernel_spmd(nc, in_maps, core_ids=list(range(NCORES)))
    out = np.zeros((B, C), np.float32)
    for r_ in res.results:
        out += r_["y"]
    return out


# revision 99
# speedup vs baseline: 1.0807x; 1.0807x over previous
"""Trainium2 Bass kernel for nn_MoEClassifier (6-layer transformer backbone +
softmax-routed MoE head), SPMD over 8 NeuronCores.

Sharding: data-parallel backbone (2 of 16 batch rows per core, params
replicated), expert-parallel MoE head (core c owns expert c) glued by an
on-device AllGather of the pooled features; the host sums the 8 per-expert
partial outputs.

v2 rewrite vs the f32r baseline:
- all weights + matmul activations in bf16 (rel-err budget is 2e-2, baseline
  was at 4e-4); weights converted on host, one large DMA per weight per layer
- no DVE reciprocal anywhere on the hot path: 1/z computed as exp(-ln z) on
  the Scalar engine (table-accurate; z > 0 always)
- LayerNorm pipelined per token-half with per-(hc,tq) hT tiles so QKV matmuls
  start as soon as their chunk is normalized
- attention softmax denominators batched per batch-row into one [NH,512] tile
- FFN W2 loop runs of-outer/m-inner so only ~3 ffT chunks are live
- weights loaded once per layer (both batch rows / token halves share them)
"""

import numpy as np
import ml_dtypes

import concourse.bass as bass
import concourse.mybir as mybir
from concourse.bass_utils import run_bass_kernel_spmd
from concourse.tile import TileContext
from concourse.vector_clock import ScopedClock

B, S, V, H, L, NH, FF, E, FE, C = 16, 512, 30522, 768, 6, 8, 3072, 8, 3072, 1000
HD = H // NH          # 96
NCORES = 8
BL = B // NCORES      # 2 batch rows per core
T = BL * S            # 1024 tokens per core
HC = H // 128         # 6 hidden chunks
FFC = FF // 128       # 24 ffn chunks
EPS = 1e-5

f32 = mybir.dt.float32
f32r = mybir.dt.float32r
bf16 = mybir.dt.bfloat16
AF = mybir.ActivationFunctionType
AX = mybir.AxisListType
OP = mybir.AluOpType
ts = bass.ts

MAX_WAITS = 1


class PatchedTileContext(TileContext):
    """Workaround for this walrus build's 1-sync-wait-per-instruction limit:
    split excess semaphore waits onto single-wait NOPs inserted immediately
    before the owning instruction (same engine, same program point)."""

    def _split_excess_waits(self, ordered):
        nc = self.nc
        for bb_name, insts in list(ordered.items()):
            new_list = []
            changed = False
            for inst in insts:
                si = getattr(inst, "sync_info", None)
                if si is not None and len(si.on_wait) > MAX_WAITS:
                    waits = list(si.on_wait)
                    movable = [
                        w for w in waits
                        if w.sync_type == "semaphore" and w.wait_mode == "sem-ge-imm"
                    ]
                    n_fixed = len(waits) - len(movable)
                    keep_n = max(0, MAX_WAITS - n_fixed)
                    n_over = max(0, len(movable) - keep_n)
                    overflow = movable[:n_over]
                    keep = [w for w in waits if w not in overflow]
                    assert len(keep) <= MAX_WAITS, (
                        f"cannot legalize waits on {inst.name}"
                    )
                    for w in overflow:
                        nop = mybir.InstNoOp(
                            name=f"I-{nc.next_id()}",
                            sync_info=mybir.SyncInfo(on_wait=[w], on_update=[]),
                            bass_nofuse=True,
                            engine=inst.engine,
                        )
                        new_list.append(nop)
                    inst.sync_info = mybir.SyncInfo(
                        on_wait=keep, on_update=list(si.on_update)
                    )
                    changed = True
                new_list.append(inst)
            if changed:
                ordered[bb_name] = new_list

    def _lower_ordered_insts(self, ordered):
        self._split_excess_waits(ordered)
        return super()._lower_ordered_insts(ordered)

    def _drain_and_barrier(self, tick_clock, wait_clock):
        nops = [self.nc.sync.nop(nofuse=True, hint=f"dw_{i}") for i in range(40)]
        drain_inst = self.nc.sync.drain()
        wait_clock.add_sem_waits(
            drain_inst.ins, ScopedClock({None: tick_clock.global_clock})
        )
        si = drain_inst.ins.sync_info
        if si is not None and len(si.on_wait) > 1:
            waits = list(si.on_wait)
            rest, keep = waits[:-1], waits[-1:]
            assert len(rest) <= len(nops)
            for nop_bi, w in zip(nops, rest):
                nop_bi.ins.sync_info = mybir.SyncInfo(on_wait=[w], on_update=[])
            drain_inst.ins.sync_info = mybir.SyncInfo(
                on_wait=keep, on_update=list(si.on_update)
            )
        self.nc.all_engine_barrier()
        assert self.sems is not None
        popped = self.nc._tile_sem_poison_stack.pop()
        assert popped is self._sem_poison
        self.nc.clear_and_free_semaphores(list(self.sems.allocated().values()))
        self.nc.all_engine_barrier()


def _r(ap):
    return ap.bitcast(f32r)


def build_program(n_layers=L, debug=False):
    nc = bass.Bass()

    x0T_d = nc.dram_tensor("x0T", [H, T], f32, kind="ExternalInput")
    wqkv_d = nc.dram_tensor("wqkv", [n_layers, H, 3 * H], bf16, kind="ExternalInput")
    # host pre-arranged [L, HD, NH, H]
    wo_d = nc.dram_tensor("wo", [n_layers, HD, NH, H], bf16, kind="ExternalInput")
    w1_d = nc.dram_tensor("w1", [n_layers, H, FF], bf16, kind="ExternalInput")
    w2_d = nc.dram_tensor("w2", [n_layers, FF, H], bf16, kind="ExternalInput")
    wr_d = nc.dram_tensor("wr", [H, E], bf16, kind="ExternalInput")
    we1_d = nc.dram_tensor("we1m", [H, FE], bf16, kind="ExternalInput")
    we2_d = nc.dram_tensor("we2m", [FE, C], bf16, kind="ExternalInput")
    maske_d = nc.dram_tensor("maske", [B, E], f32, kind="ExternalInput")
    # consts: col0 = -1/H, col1 = 1.0, col2.. = 1.0 row for broadcasts
    ones_d = nc.dram_tensor("ones", [128, 128], f32, kind="ExternalInput")
    # col0 = -1/H, col1 = EPS
    negh_d = nc.dram_tensor("negh", [128, 2], f32, kind="ExternalInput")
    neghb_d = nc.dram_tensor("neghb", [128, 1], bf16, kind="ExternalInput")
    onesb_d = nc.dram_tensor("onesb", [128, 128], bf16, kind="ExternalInput")
    id128_d = nc.dram_tensor("id128", [128, 128], f32, kind="ExternalInput")
    id16_d = nc.dram_tensor("id16", [16, 16], f32, kind="ExternalInput")
    y_d = nc.dram_tensor("y", [B, C], f32, kind="ExternalOutput")

    dbg = {}
    if debug:
        for name, shape in [("dbg_h1", [H, T]), ("dbg_xa", [H, T]),
                            ("dbg_x1", [H, T]), ("dbg_pool", [BL, H]),
                            ("dbg_gate", [B, E])]:
            dbg[name] = nc.dram_tensor(name, shape, f32, kind="ExternalOutput")

    from contextlib import ExitStack

    lp = nc.allow_low_precision(reason="bf16 matmuls + f32r stats")
    lp.__enter__()
    with PatchedTileContext(nc) as tc:
        with tc.tile_pool(name="sbc", bufs=1) as sbc:
            bk = ExitStack()
            sbw = bk.enter_context(tc.tile_pool(name="sbw", bufs=1))
            sbw3 = bk.enter_context(tc.tile_pool(name="sbw3", bufs=2))
            sbh = bk.enter_context(tc.tile_pool(name="sbh", bufs=12))
            sba = bk.enter_context(tc.tile_pool(name="sba", bufs=1))
            sbs = bk.enter_context(tc.tile_pool(name="sbs", bufs=2))
            sbr = bk.enter_context(tc.tile_pool(name="sbr", bufs=4))
            sbx = bk.enter_context(tc.tile_pool(name="sbx", bufs=7))
            sbf = bk.enter_context(tc.tile_pool(name="sbf", bufs=10))
            sbhf = bk.enter_context(tc.tile_pool(name="sbhf", bufs=3))
            ps_st = bk.enter_context(
                tc.tile_pool(name="ps_st", bufs=2, space="PSUM"))
            ps_bc = bk.enter_context(
                tc.tile_pool(name="ps_bc", bufs=2, space="PSUM"))

            # ---------------- constants
            negh = sbc.tile([128, 1], bf16, tag="negh")     # -1/H
            nc.sync.dma_start(negh[:], neghb_d[:])
            epsc = sbc.tile([128, 1], f32, tag="epsc")      # EPS
            nc.sync.dma_start(epsc[:], negh_d[:, 1:2])
            # ones rows at partitions 0 and 32 (broadcast-matmul lhsT must
            # share base_partition with its rhs row)
            onesrow = sbc.tile([33, 128], bf16, tag="onesrow")
            nc.sync.dma_start(onesrow[:], onesb_d[0:33, :])
            onesb = sbc.tile([128, 32], bf16, tag="onesb")    # bf16 1.0
            nc.sync.dma_start(onesb[:], onesb_d[:, 0:32])
            id128 = sbc.tile([128, 128], f32, tag="id128")
            nc.sync.dma_start(id128[:], id128_d[:])

            x = sbc.tile([128, HC, T], f32, tag="x")
            for hc in range(HC):
                nc.sync.dma_start(
                    x[:, hc, :],
                    x0T_d.rearrange("(hc p) t -> p hc t", p=128)[:, hc, :])

            def layer_norm_half(tq, hts, pool=None, dt=bf16):
                """LN over hidden for token half tq: appends 6 normalized
                chunk tiles to hts. Uses the static ps_st/ps_bc PSUM pools;
                all matmuls bf16 (stats from a bf16 copy of x)."""
                pool_ = pool if pool is not None else sbh
                sx = ps_st.tile([1, 512], f32, tag="st")
                sq = ps_st.tile([1, 512], f32, tag="st")
                xbs = []
                for hc in range(HC):
                    xb = sbx.tile([128, 512], bf16, tag="xb")
                    nc.any.tensor_copy(xb[:], x[:, hc, ts(tq, 512)])
                    xbs.append(xb)
                    sqc = sbs.tile([128, 512], bf16, tag="sqc")
                    nc.scalar.activation(sqc[:], x[:, hc, ts(tq, 512)], AF.Square,
                                         scale=float(1.0 / np.sqrt(H)))
                    nc.tensor.matmul(sx[:], negh[:], xb[:],
                                     start=(hc == 0), stop=(hc == HC - 1))
                    nc.tensor.matmul(sq[:], onesb[:, 0:1], sqc[:],
                                     start=(hc == 0), stop=(hc == HC - 1))
                # row math: nmu = sx (= -mu); var = sq - nmu^2; r = exp(-.5 ln(var+eps))
                nmu = sbr.tile([1, 512], bf16, tag="rowb")
                nc.vector.tensor_copy(nmu[:], sx[:])
                mu2 = sbr.tile([1, 512], f32, tag="row")
                nc.vector.tensor_tensor(mu2[:], nmu[:], nmu[:], OP.mult)
                var = sbr.tile([1, 512], f32, tag="row")
                nc.vector.tensor_tensor(var[:], sq[:], mu2[:], OP.subtract)
                lnv = sbr.tile([1, 512], f32, tag="row")
                nc.scalar.activation(lnv[:], var[:], AF.Ln, bias=epsc[0:1])
                rstd = sbr.tile([1, 512], bf16, tag="rowb")
                nc.scalar.activation(rstd[:], lnv[:], AF.Exp, scale=-0.5)
                nb = ps_bc.tile([128, 512], f32, tag="bc")
                rb = ps_bc.tile([128, 512], f32, tag="bc")
                nc.tensor.matmul(nb[:], onesrow[0:1, :], nmu[:],
                                 start=True, stop=True)
                nc.tensor.matmul(rb[:], onesrow[0:1, :], rstd[:],
                                 start=True, stop=True)
                nbS = sbs.tile([128, 512], bf16, tag="nbS")
                nc.scalar.activation(nbS[:], nb[:], AF.Copy)
                rbS = sbs.tile([128, 512], bf16, tag="nbS")
                nc.scalar.activation(rbS[:], rb[:], AF.Copy)
                for hc in range(HC):
                    tmp = sbs.tile([128, 512], bf16, tag="tmp")
                    nc.vector.tensor_tensor(tmp[:], xbs[hc][:], nbS[:], OP.add)
                    ht = pool_.tile([128, 512], dt, tag="hT")
                    nc.vector.tensor_tensor(ht[:], tmp[:], rbS[:], OP.mult)
                    hts.append(ht)

            pooledT = sbc.tile([128, HC, BL], f32, tag="pooledT")
            hts = []
            layer_norm_half(0, hts)
            layer_norm_half(1, hts)
            for l in range(n_layers):
                # ---------------- weights for this layer (one DMA each)
                # issue the big per-layer prefetches on the second HWDGE ring
                # (ACT sequencer) so the FFN weight-quarter stream on the sync
                # ring never queues behind them
                wqkv = sbw.tile([128, HC, 3 * H], bf16, tag="wqkv")
                nc.scalar.dma_start(
                    wqkv[:], wqkv_d[l].rearrange("(hc p) m -> p hc m", p=128))
                wo_t = sbw.tile([HD, NH, H], bf16, tag="wo")
                nc.scalar.dma_start(wo_t[:], wo_d[l])

                # hts for this layer were produced at the tail of the previous
                # layer's FFN (software-pipelined LN1)
                for b2 in range(BL):
                    ht_b = hts[b2 * HC:(b2 + 1) * HC]
                    qT = sba.tile([HD, NH, 512], bf16, tag="qT")
                    kT = sba.tile([HD, NH, 512], bf16, tag="kT")
                    v_aug = sba.tile([128, 4, NH, HD + 1], bf16, tag="vaug")
                    nc.vector.tensor_copy(
                        v_aug[:, :, :, HD:],
                        onesb[:].rearrange("p (a b c) -> p a b c", a=4, b=NH, c=1))
                    with tc.tile_pool(name=f"psqkv_{l}_{b2}", bufs=4,
                                      space="PSUM") as ps:
                        for h in range(NH):
                            pq = ps.tile([HD, 512], f32, tag="mm")
                            pk = ps.tile([HD, 512], f32, tag="mm")
                            for hc in range(HC):
                                rhs = ht_b[hc][:]
                                nc.tensor.matmul(
                                    pq[:], wqkv[:, hc, h * HD:(h + 1) * HD], rhs,
                                    start=(hc == 0), stop=(hc == HC - 1))
                                nc.tensor.matmul(
                                    pk[:], wqkv[:, hc, H + h * HD:H + (h + 1) * HD],
                                    rhs, start=(hc == 0), stop=(hc == HC - 1))
                            nc.vector.tensor_copy(qT[:, h, :], pq[:])
                            nc.scalar.activation(kT[:, h, :], pk[:], AF.Copy)
                        # V: token-major via lhsT = hT chunks
                        for tt in range(4):
                            pv0 = ps.tile([128, 384], f32, tag="mm")
                            pv1 = ps.tile([128, 384], f32, tag="mm")
                            for hc in range(HC):
                                lhs = ht_b[hc][:, ts(tt, 128)]
                                nc.tensor.matmul(
                                    pv0[:], lhs, wqkv[:, hc, 2 * H:2 * H + 384],
                                    start=(hc == 0), stop=(hc == HC - 1))
                                nc.tensor.matmul(
                                    pv1[:], lhs, wqkv[:, hc, 2 * H + 384:3 * H],
                                    start=(hc == 0), stop=(hc == HC - 1))
                            nc.vector.tensor_copy(
                                v_aug[:, tt, 0:4, :HD],
                                pv0[:].rearrange("p (h d) -> p h d", h=4))
                            nc.vector.tensor_copy(
                                v_aug[:, tt, 4:8, :HD],
                                pv1[:].rearrange("p (h d) -> p h d", h=4))

                    # attention; 1/z = exp(-ln z) on ScalarE, batched per
                    # head-pair to halve the ACT denominator work
                    oT = sba.tile([HD, NH, 512], bf16, tag="oT")
                    with tc.tile_pool(name=f"psat_{l}_{b2}", bufs=2,
                                      space="PSUM") as ps:
                        for h in range(NH):
                            expT = sbs.tile([128, 4, 512], bf16, tag="expT")
                            for tk in range(4):
                                psc = ps.tile([128, 512], f32, tag="sc")
                                nc.tensor.matmul(
                                    psc[:], kT[:, h, ts(tk, 128)], qT[:, h, :],
                                    start=True, stop=True)
                                nc.scalar.activation(
                                    expT[:, tk, :], psc[:], AF.Exp,
                                    scale=float(1.0 / np.sqrt(HD)))
                            po = ps.tile([HD + 1, 512], f32, tag="o")
                            for tk in range(4):
                                nc.tensor.matmul(po[:], v_aug[:, tk, h, :],
                                                 expT[:, tk, :],
                                                 start=(tk == 0), stop=(tk == 3))
                            lnz = sbr.tile([1, 512], f32, tag="row")
                            nc.scalar.activation(lnz[:], po[HD:HD + 1, :], AF.Ln)
                            rz = sbr.tile([1, 512], bf16, tag="rowb")
                            nc.scalar.activation(rz[:], lnz[:], AF.Exp,
                                                 scale=-1.0)
                            prb = ps_bc.tile([HD, 512], f32, tag="bc")
                            nc.tensor.matmul(prb[:], onesrow[0:1, :HD], rz[:],
                                             start=True, stop=True)
                            rbS = sbs.tile([HD, 512], bf16, tag="rbS")
                            nc.vector.tensor_copy(rbS[:], prb[:])
                            nc.vector.tensor_tensor(oT[:, h, :], po[:HD, :],
                                                    rbS[:], OP.mult)

                    # Wo + residual
                    with tc.tile_pool(name=f"pswo_{l}_{b2}", bufs=2,
                                      space="PSUM") as ps:
                        for m in range(HC):
                            pwo = ps.tile([128, 512], f32, tag="wo")
                            for h in range(NH):
                                nc.tensor.matmul(pwo[:], wo_t[:, h, ts(m, 128)],
                                                 oT[:, h, :],
                                                 start=(h == 0),
                                                 stop=(h == NH - 1))
                            nc.vector.tensor_tensor(x[:, m, ts(b2, 512)],
                                                    x[:, m, ts(b2, 512)],
                                                    pwo[:], OP.add)

                if debug and l == 0:
                    nc.sync.dma_start(
                        dbg["dbg_xa"].rearrange("(hc p) t -> p hc t", p=128), x[:])

                # ---------------- LN2 + FFN (LN1 of layer l+1 interleaved)
                hts_next = []
                for tq in range(2):
                    hts2 = []
                    layer_norm_half(tq, hts2)
                    with tc.tile_pool(name=f"psf1_{l}_{tq}", bufs=2,
                                      space="PSUM") as psw1, \
                         tc.tile_pool(name=f"psf2_{l}_{tq}", bufs=2,
                                      space="PSUM") as psx2:
                        for t4 in range(4):
                            w1_4 = sbw3.tile([128, HC, 768], bf16, tag="w1h")
                            nc.sync.dma_start(
                                w1_4[:],
                                w1_d[l].rearrange("(hc p) m -> p hc m",
                                                  p=128)[:, :, ts(t4, 768)])
                            w2_4 = sbw3.tile([128, 6, H], bf16, tag="w2h")
                            nc.sync.dma_start(
                                w2_4[:],
                                w2_d[l].rearrange(
                                    "(fc p) m -> p fc m",
                                    p=128)[:, t4 * 6:(t4 + 1) * 6, :])
                            ffTs = []
                            for of6 in range(6):
                                pf = psw1.tile([128, 512], f32, tag="w1")
                                for hc in range(HC):
                                    nc.tensor.matmul(
                                        pf[:], w1_4[:, hc, ts(of6, 128)],
                                        hts2[hc][:],
                                        start=(hc == 0), stop=(hc == HC - 1))
                                ffT = sbf.tile([128, 512], bf16, tag="ffT")
                                nc.scalar.activation(ffT[:], pf[:], AF.Gelu)
                                ffTs.append(ffT)
                            for m in range(HC):
                                px2 = psx2.tile([128, 512], f32, tag="x2")
                                for of6 in range(6):
                                    nc.tensor.matmul(
                                        px2[:], w2_4[:, of6, ts(m, 128)],
                                        ffTs[of6][:],
                                        start=(of6 == 0), stop=(of6 == 5))
                                nc.vector.tensor_tensor(x[:, m, ts(tq, 512)],
                                                        x[:, m, ts(tq, 512)],
                                                        px2[:], OP.add)
                    # next layer's LN1 for this token half: its stats/normalize
                    # interleave with the other half's FFN matmul stream
                    if l < n_layers - 1:
                        layer_norm_half(tq, hts_next)
                hts = hts_next
                if debug and l == 0:
                    nc.sync.dma_start(
                        dbg["dbg_x1"].rearrange("(hc p) t -> p hc t", p=128), x[:])

            # ---------------- final LN + pooling (f32 for gate fidelity)
            for tq in range(2):
                htf = []
                layer_norm_half(tq, htf, pool=sbhf, dt=f32)
                for hc in range(HC):
                    acc = sbr.tile([128, 1], f32, tag="poolacc")
                    nc.vector.reduce_sum(acc[:], htf[hc][:], axis=AX.X)
                    nc.vector.tensor_scalar_mul(pooledT[:, hc, tq:tq + 1],
                                                acc[:], 1.0 / S)
            pool_tok = sbc.tile([BL, H], f32, tag="pool_tok")
            with tc.tile_pool(name="pstr", bufs=2, space="PSUM") as ps:
                for hc in range(HC):
                    pt = ps.tile([BL, 128], f32, tag="tr")
                    nc.tensor.transpose(pt[:], pooledT[:, hc, :], id128[:])
                    nc.vector.tensor_copy(pool_tok[:, ts(hc, 128)], pt[:])
            if debug:
                nc.sync.dma_start(dbg["dbg_pool"][:], pool_tok[:])

            # release backbone pools so the head weights fit in SBUF
            bk.close()

            # ---------------- AllGather (in-context) + MoE head
            with tc.tile_pool(name="dcc", bufs=1, space="DRAM") as dcc, \
                 tc.tile_pool(name="hsb1", bufs=1) as hb1, \
                 tc.tile_pool(name="hsb4", bufs=4) as hb4:
                in_b = dcc.tile([BL, H], f32, tag="ccin")
                out_b = dcc.tile([B, H], f32, tag="ccout", addr_space="Shared")
                nc.sync.dma_start(in_b[:], pool_tok[:])
                nc.gpsimd.collective_compute(
                    "AllGather", OP.bypass,
                    replica_groups=[list(range(NCORES))],
                    ins=[in_b.opt()], outs=[out_b.opt()],
                )

                # weight DMAs (chunked so first matmuls start early; they
                # overlap the collective)
                we1 = hb1.tile([128, HC, FE], bf16, tag="we1")
                for fr in range(3):
                    nc.sync.dma_start(
                        we1[:, :, ts(fr, 1024)],
                        we1_d.rearrange("(hc p) m -> p hc m",
                                        p=128)[:, :, ts(fr, 1024)])
                we2 = hb1.tile([128, FFC, C], bf16, tag="we2")
                for fr in range(3):
                    nc.sync.dma_start(
                        we2[:, fr * 8:(fr + 1) * 8, :],
                        we2_d.rearrange("(fc p) m -> p fc m",
                                        p=128)[:, fr * 8:(fr + 1) * 8, :])
                wr_t = hb1.tile([128, HC, E], bf16, tag="wr")
                nc.sync.dma_start(wr_t[:],
                                  wr_d.rearrange("(hc p) e -> p hc e", p=128))
                id16 = hb1.tile([16, 16], f32, tag="id16")
                nc.sync.dma_start(id16[:], id16_d[:])
                maske = hb1.tile([B, E], f32, tag="maske")
                nc.sync.dma_start(maske[:], maske_d[:])

                pg = hb1.tile([B, H], f32, tag="pg")
                nc.gpsimd.dma_start(pg[:], out_b[:])
                paT = hb1.tile([128, HC, B], bf16, tag="paT")
                hps_cm = tc.tile_pool(name="hps", bufs=2, space="PSUM")
                ps = hps_cm.__enter__()
                for hc in range(HC):
                    pt = ps.tile([128, B], f32, tag="tr")
                    nc.tensor.transpose(pt[:], pg[:, ts(hc, 128)], id16[:])
                    nc.vector.tensor_copy(paT[:, hc, :], pt[:])
                # gate (token-major [B, E])
                pgl = ps.tile([B, E], f32, tag="gl")
                for hc in range(HC):
                    nc.tensor.matmul(pgl[:], paT[:, hc, :], wr_t[:, hc, :],
                                     start=(hc == 0), stop=(hc == HC - 1))
                gate = hb1.tile([B, E], f32, tag="gate")
                gmax = hb4.tile([B, 1], f32, tag="grow")
                nc.vector.reduce_max(gmax[:], pgl[:], axis=AX.X)
                ngmax = hb4.tile([B, 1], f32, tag="grow")
                nc.vector.tensor_scalar_mul(ngmax[:], gmax[:], -1.0)
                nc.scalar.activation(gate[:], pgl[:], AF.Exp, bias=ngmax[:])
                gsum = hb4.tile([B, 1], f32, tag="grow")
                nc.vector.reduce_sum(gsum[:], gate[:], axis=AX.X)
                grecip = hb4.tile([B, 1], f32, tag="grow")
                nc.vector.reciprocal(grecip[:], gsum[:])
                nc.vector.tensor_scalar_mul(gate[:], gate[:], grecip[:])
                if debug:
                    nc.sync.dma_start(dbg["dbg_gate"][:], gate[:])
                gcol = hb1.tile([B, 1], f32, tag="gcol")
                nc.vector.tensor_tensor(maske[:], gate[:], maske[:], OP.mult)
                nc.vector.reduce_sum(gcol[:], maske[:], axis=AX.X)

                # ehT = gelu(We1^T @ pooled_all) feature-major [FE, B]
                ehT = hb1.tile([128, FFC, B], bf16, tag="ehT")
                for fet in range(FFC):
                    pe_ = ps.tile([128, B], f32, tag="eh")
                    for hc in range(HC):
                        nc.tensor.matmul(pe_[:], we1[:, hc, ts(fet, 128)],
                                         paT[:, hc, :],
                                         start=(hc == 0), stop=(hc == HC - 1))
                    nc.scalar.activation(ehT[:, fet, :], pe_[:], AF.Gelu)
                # elog token-major [B, C] scaled by this expert's gate column
                y_sb = hb1.tile([B, C], f32, tag="y")
                for cn in range(2):
                    csz = C // 2
                    pel = ps.tile([B, csz], f32, tag="el")
                    for fet in range(FFC):
                        nc.tensor.matmul(pel[:], ehT[:, fet, :],
                                         we2[:, fet, ts(cn, csz)],
                                         start=(fet == 0), stop=(fet == FFC - 1))
                    nc.vector.tensor_scalar_mul(y_sb[:, ts(cn, csz)], pel[:],
                                                gcol[:])
                hps_cm.__exit__(None, None, None)
                nc.sync.dma_start(y_d[:], y_sb[:])

    lp.__exit__(None, None, None)
    return nc, dbg


_CACHE = {}


def _get_program(n_layers=L, debug=False):
    key = (n_layers, debug)
    if key not in _CACHE:
        _CACHE[key] = build_program(n_layers, debug)
    return _CACHE[key]


def prepare_inputs(inputs, n_layers=L):
    """Host-side shard prep: embedding gather, bf16 weight conversion,
    per-core slicing, asserts."""
    ids = np.asarray(inputs["input_ids"])
    mask = np.asarray(inputs["attention_mask"])
    assert (mask == 1).all(), "kernel assumes attention_mask == ones"
    for k in ("bqkv", "bo", "b1", "b2", "br", "be1", "be2",
              "ln1_b", "ln2_b", "lnf_b"):
        assert not np.any(np.asarray(inputs[k])), f"{k} must be zero"
    for k in ("ln1_g", "ln2_g", "lnf_g"):
        assert np.all(np.asarray(inputs[k]) == 1.0), f"{k} must be ones"

    bf = ml_dtypes.bfloat16
    tok = np.asarray(inputs["tok_emb"], np.float32)
    pos = np.asarray(inputs["pos_emb"], np.float32)
    x0 = tok[ids] + pos[None]                      # [B, S, H]
    wqkv = np.ascontiguousarray(
        np.asarray(inputs["Wqkv"], np.float32)[:n_layers]).astype(bf)
    wo = np.asarray(inputs["Wo"], np.float32)[:n_layers]
    # [L, H, H] -> [L, HD, NH, H]
    wo = np.ascontiguousarray(
        wo.reshape(n_layers, NH, HD, H).transpose(0, 2, 1, 3)).astype(bf)
    w1 = np.ascontiguousarray(
        np.asarray(inputs["W1"], np.float32)[:n_layers]).astype(bf)
    w2 = np.ascontiguousarray(
        np.asarray(inputs["W2"], np.float32)[:n_layers]).astype(bf)
    wr = np.ascontiguousarray(np.asarray(inputs["Wr"], np.float32)).astype(bf)
    we1 = np.asarray(inputs["We1"], np.float32)
    we2 = np.asarray(inputs["We2"], np.float32)
    id128 = np.eye(128, dtype=np.float32)
    id16 = np.eye(16, dtype=np.float32)
    negh = np.stack([np.full(128, -1.0 / H, np.float32),
                     np.full(128, EPS, np.float32)], axis=1)
    neghb = np.full((128, 1), -1.0 / H, np.float32).astype(bf)
    onesb = np.ones((128, 128), bf)

    in_maps = []
    for c in range(NCORES):
        rows = x0[c * BL:(c + 1) * BL]              # [BL, S, H]
        x0T = np.ascontiguousarray(rows.reshape(T, H).T)   # [H, T]
        maske = np.zeros((B, E), np.float32)
        maske[:, c] = 1.0
        in_maps.append({
            "x0T": x0T, "wqkv": wqkv, "wo": wo, "w1": w1, "w2": w2,
            "wr": wr, "we1m": np.ascontiguousarray(we1[c]).astype(bf),
            "we2m": np.ascontiguousarray(we2[c]).astype(bf),
            "maske": maske, "id128": id128, "id16": id16,
            "ones": np.ones((128, 128), np.float32),
            "negh": negh, "neghb": neghb, "onesb": onesb,
        })
    return in_maps


def kernel(**inputs):
    nc, _dbg = _get_program(L, debug=False)
    in_maps = prepare_inputs(inputs, L)
    res = run_bass_kernel_spmd(nc, in_maps, core_ids=list(range(NCORES)))
    out = np.zeros((B, C), np.float32)
    for r_ in res.results:
        out += r_["y"]
    return out
